# revision 20
# baseline (speedup 1.0000x reference)
"""Causal self-attention (B=2, T=2048, C=1024, H=16) on 8 Trainium2 NeuronCores.

Sharding: tensor-parallel over heads only — each core owns 2 heads for BOTH
batches.  This makes the SPMD program fully uniform: after attention, one
8-way AllToAll redistributes head-sharded attention outputs (yT) into
token-sharded full-channel blocks, and each core runs the output projection
for its (batch, 512-token) slice.  No core-dependent addressing anywhere;
the host assembles the 8 output slices.

Compute dtype: bf16 on TensorE with fp32 PSUM accumulation (validated at
~3.5e-3 rel err vs the fp32 reference).

Kernel layout choices:
- x is shipped pre-transposed and pre-tiled (xT [NQ, KC, 128, TQ] bf16) so
  the kqv projection produces K^T/Q^T/V^T directly ([dim, T], dims on
  partitions) and can start as soon as the first column-chunk lands.
- kqv bias is folded into the PSUM->SBUF evacuation (DVE tensor_scalar).
- Scores are computed transposed, sT[k, q] = kT_blk.T @ qT, so softmax's
  denominator folds into the AV matmul as an extra ones-column of V
  (lhsT = [v | ones] -> row 64 of yT accumulates sum_k exp).
- V is re-laid-out [k, d] via PE transposes of vT.
- Division by the denominator: reciprocal_approx_fast (DVE) +
  partition_broadcast (GPSIMD) + tensor_mul (DVE).
- Causal masking: upper-triangle-only score blocks are never computed; the
  diagonal blocks get a tril mask via gpsimd.affine_select (free engine).
- Consecutive unmasked score blocks are computed in pairs into a 2-bank
  PSUM tile so each ACT exp call covers 1024 columns (ACT is the
  second-busiest engine).
- Output projection folds its bias via a ones-row (K=1 chunk).
- Emission interleaves batch 1's kqv with batch 0's attention to keep the
  TensorEngine dense (HAM stays warm).
"""

import hashlib
import numpy as np
import ml_dtypes

B, T, C, H = 2, 2048, 1024, 16
HD = C // H            # 64
NCORES = 8
TQ = 512               # q-chunk width
NJ = T // 128          # 16 k-blocks
NQ = T // TQ           # 4 q-chunks
KC = C // 128          # 8 contraction chunks

bfloat16 = ml_dtypes.bfloat16


# ---------------------------------------------------------------- schedules
def _make_schedule(att_mask):
    """Per q-chunk list of (j, n_off, n_len, masked).

    masked is None (no mask), 'tril' (apply causal tril to slab cols 0:128),
    or an int index into the general mask table.
    """
    m = np.asarray(att_mask).reshape(T, T)
    tril = np.tril(np.ones((T, T), m.dtype))
    if np.array_equal(m, tril):
        sched = []
        for Q in range(NQ):
            ent = [(j, 0, TQ, None) for j in range(4 * Q)]
            for j in range(4 * Q, 4 * Q + 4):
                n_off = 128 * (j - 4 * Q)
                ent.append((j, n_off, TQ - n_off, "tril"))
            sched.append(ent)
        return sched, None

    masks = []
    mask_ids = {}
    sched = []
    for Q in range(NQ):
        ent = []
        for j in range(NJ):
            blk = m[Q * TQ:(Q + 1) * TQ, j * 128:(j + 1) * 128].T  # [128k,512q]
            if not blk.any():
                continue
            if blk.all():
                ent.append((j, 0, TQ, None))
                continue
            key = blk.tobytes()
            if key not in mask_ids:
                mask_ids[key] = len(masks)
                masks.append(blk.astype(np.float32))
            ent.append((j, 0, TQ, mask_ids[key]))
        sched.append(ent)
    masks = np.stack(masks) if masks else None
    return sched, masks


def _group_entries(ents):
    """Group consecutive full-width unmasked entries into pairs."""
    groups = []
    i = 0
    while i < len(ents):
        j, n_off, n_len, mid = ents[i]
        if (mid is None and n_len == TQ and i + 1 < len(ents)
                and ents[i + 1][3] is None and ents[i + 1][2] == TQ):
            groups.append(("pair", ents[i], ents[i + 1]))
            i += 2
        else:
            groups.append(("single", ents[i]))
            i += 1
    return groups


def _sched_key(sched, masks):
    h = hashlib.sha256(repr(sched).encode())
    if masks is not None:
        h.update(masks.tobytes())
    return h.hexdigest()


# ---------------------------------------------------------------- builder
_BUILD_CACHE = {}


def _build(sched, masks):
    from concourse import bacc, tile, mybir
    from concourse.masks import make_identity

    BF16, F32 = mybir.dt.bfloat16, mybir.dt.float32
    n_masks = 0 if masks is None else masks.shape[0]

    nc = bacc.Bacc("TRN2", target_bir_lowering=False, debug=False,
                   num_devices=NCORES)

    # -------- I/O ----------------------------------------------------------
    xT_d = [nc.dram_tensor(f"xT{b}", [NQ, KC, 128, TQ], BF16,
                           kind="ExternalInput") for b in range(B)]
    wk_d = nc.dram_tensor("wk", [C, 6 * HD], BF16, kind="ExternalInput")
    bk_d = nc.dram_tensor("bk", [128, 3], F32, kind="ExternalInput")
    wp_d = nc.dram_tensor("wp", [C, C], BF16, kind="ExternalInput")
    bp_d = nc.dram_tensor("bp", [1, C], BF16, kind="ExternalInput")
    if n_masks:
        mk_d = nc.dram_tensor("mk", [n_masks * 128, TQ], BF16,
                              kind="ExternalInput")
    out_d = nc.dram_tensor("out", [TQ, C], F32, kind="ExternalOutput")

    VW = 2 * HD + 2     # v_ext tile width: [vA | onesA | vB | onesB] = 130

    with tile.TileContext(nc) as tc:
        with tc.tile_pool(name="big", bufs=1) as big, \
             tc.tile_pool(name="work", bufs=1) as work, \
             tc.tile_pool(name="pmm", bufs=2, space="PSUM") as pmm, \
             tc.tile_pool(name="pqk", bufs=2, space="PSUM") as pqk, \
             tc.tile_pool(name="pyt", bufs=2, space="PSUM") as pyt:

            # ---- persistent SBUF tensors ----------------------------------
            wk = [big.tile([128, 6 * HD], BF16, name=f"wk{k}", tag=f"wk{k}")
                  for k in range(KC)]
            bkp = big.tile([128, 3], F32, name="bkp", tag="bkp")
            xT = [[big.tile([128, T], BF16, name=f"xT{b}_{k}", tag=f"xT{b}_{k}")
                   for k in range(KC)] for b in range(B)]
            ones_r = big.tile([1, T], BF16, name="ones_r", tag="ones_r")
            wp = [big.tile([128, C], BF16, name=f"wp{k}", tag=f"wp{k}")
                  for k in range(KC)]
            bp = big.tile([1, C], BF16, name="bp", tag="bp")
            ident = big.tile([128, 128], BF16, name="ident", tag="ident")

            # input DMAs: wk first, then xT0 (tiled), wp, xT1 — issued
            # round-robin across engines so descriptor generation (the
            # DIRECT2D writes on the issuing sequencer) parallelizes
            _eng = [nc.sync, nc.scalar, nc.gpsimd]
            _ei = [0]

            def _dma(out, in_):
                _eng[_ei[0] % len(_eng)].dma_start(out=out, in_=in_)
                _ei[0] += 1

            for k in range(KC):
                _dma(wk[k][:, :], wk_d.ap()[k * 128:(k + 1) * 128, :])
            _dma(bkp[:, :], bk_d.ap())
            for b in range(B):
                for n in range(NQ):
                    for k in range(KC):
                        _dma(xT[b][k][:, n * TQ:(n + 1) * TQ],
                             xT_d[b].ap()[n, k, :, :])
                if b == 0:
                    # wp/bp are needed only by the projection, but issue
                    # their DMAs early so they aren't queued behind the
                    # dependency-stalled AllToAll DMAs at the end
                    for k in range(KC):
                        _dma(wp[k][:, :], wp_d.ap()[k * 128:(k + 1) * 128, :])
                    _dma(bp[:, :], bp_d.ap())
            nc.vector.memset(ones_r[:, :], 1.0)
            make_identity(nc, ident[:, :])

            if n_masks:
                mks = big.tile([128, n_masks * TQ], BF16, name="mks",
                               tag="mks")
                for i in range(n_masks):
                    nc.sync.dma_start(out=mks[:, i * TQ:(i + 1) * TQ],
                                      in_=mk_d.ap()[i * 128:(i + 1) * 128, :])

            # per-batch attention tensors
            kT = [big.tile([128, T], BF16, name=f"kT{b}", tag=f"kT{b}")
                  for b in range(B)]
            qT = [big.tile([128, T], BF16, name=f"qT{b}", tag=f"qT{b}")
                  for b in range(B)]
            vT = [big.tile([128, T], BF16, name=f"vT{b}", tag=f"vT{b}")
                  for b in range(B)]
            vx = [big.tile([128, NJ * VW], BF16, name=f"vx{b}", tag=f"vx{b}")
                  for b in range(B)]
            yT = [big.tile([128, T], BF16, name=f"yT{b}", tag=f"yT{b}")
                  for b in range(B)]

            for b in range(B):
                vx_v = vx[b].rearrange("p (t c) -> p t c", t=NJ)
                nc.vector.memset(vx_v[:, :, HD::HD + 1], 1.0)

            dst = {0: kT, 1: qT, 2: vT}

            def kqv_steps(b):
                # kqvT[m-tile] = wk[:,m].T @ xT; bias folded into evacuation
                for m in range(3):
                    for n in range(NQ):
                        ps = pmm.tile([128, TQ], F32, name="kqv_ps", tag="mm",
                                      bufs=2)
                        for k in range(KC):
                            nc.tensor.matmul(
                                ps[:, :],
                                wk[k][:, m * 128:(m + 1) * 128],
                                xT[b][k][:, n * TQ:(n + 1) * TQ],
                                start=(k == 0), stop=(k == KC - 1))
                        nc.vector.tensor_scalar_add(
                            dst[m][b][:, n * TQ:(n + 1) * TQ], ps[:, :],
                            bkp[:, m:m + 1])
                        yield

            def transpose_steps(b):
                vx_v = vx[b].rearrange("p (t c) -> p t c", t=NJ)
                for t in range(NJ):
                    tr = pmm.tile([128, 128], BF16, name="tr_ps", tag="mm",
                                  bufs=2)
                    nc.tensor.transpose(tr[:, :],
                                        vT[b][:, t * 128:(t + 1) * 128],
                                        ident[:, :])
                    o = vx_v[:, t, :].rearrange("p (u c) -> p u c", u=2)
                    nc.vector.tensor_copy(
                        o[:, :, 0:HD], tr.rearrange("p (u c) -> p u c", u=2))
                    if t % 4 == 3:
                        yield

            def attn_steps(b, Q):
                ents = sched[Q]
                if not ents:
                    for t in range(2):
                        nc.vector.memset(
                            yT[b][HD * t:HD * (t + 1), Q * TQ:(Q + 1) * TQ],
                            0.0)
                    return
                yps = [pyt.tile([HD + 1, TQ], F32, name=f"y_ps{t}", tag="yt",
                                bufs=2) for t in range(2)]
                n_av = {0: 0, 1: 0}   # AV matmuls emitted so far per head
                tot = sum(2 if (mid == "tril" and n_len > 128) else 1
                          for (j, n_off, n_len, mid) in ents)
                total_av = {0: tot, 1: tot}

                def emit_avs(avs):
                    # avs: list of (t, j, src, o_off, o_len)
                    for t, j, src, o_off, o_len in avs:
                        first = n_av[t] == 0
                        last = n_av[t] == total_av[t] - 1
                        nc.tensor.matmul(
                            yps[t][:, o_off:o_off + o_len],
                            vx[b][:, j * VW + t * (HD + 1):
                                  j * VW + (t + 1) * (HD + 1)],
                            src, start=first, stop=last,
                            skip_group_check=True)
                        n_av[t] += 1

                # Per entry: both heads' QK matmuls back-to-back (they hit
                # disjoint PE row-groups and can run concurrently), then the
                # exps, then the previous entry's AV matmuls (one-entry lag
                # so the TensorE never waits on ACT).
                pending = []
                pending2 = []
                for j, n_off, n_len, mid in ents:
                    cur = []
                    sps = []
                    for t in range(2):
                        sp = pqk.tile([128, TQ], F32, name="s_ps",
                                      tag="qk", bufs=4)
                        nc.tensor.matmul(
                            sp[:, 0:n_len],
                            kT[b][HD * t:HD * (t + 1),
                                  j * 128:(j + 1) * 128],
                            qT[b][HD * t:HD * (t + 1),
                                  Q * TQ + n_off:(Q + 1) * TQ],
                            start=True, stop=True, skip_group_check=True)
                        sps.append(sp)
                    for t in range(2):
                        sp = sps[t]
                        slab = work.tile([128, TQ], BF16, name="slab",
                                         tag="slab", bufs=8)
                        nc.scalar.activation(
                            slab[:, 0:n_len], sp[:, 0:n_len],
                            mybir.ActivationFunctionType.Exp, scale=0.125)
                        if mid == "tril":
                            slab2 = work.tile([128, 128], BF16,
                                              name="slab2", tag="slab2",
                                              bufs=6)
                            nc.gpsimd.affine_select(
                                out=slab2[:, :], in_=slab[:, 0:128],
                                compare_op=mybir.AluOpType.is_ge,
                                fill=0.0, base=0, pattern=[[1, 128]],
                                channel_multiplier=-1)
                            cur.append((t, j, slab2[:, :], n_off, 128))
                            if n_len > 128:
                                cur.append((t, j, slab[:, 128:n_len],
                                            n_off + 128, n_len - 128))
                        elif mid is not None:
                            slab2 = work.tile([128, TQ], BF16,
                                              name="slab2m", tag="slab2m",
                                              bufs=4)
                            nc.vector.tensor_mul(
                                slab2[:, 0:n_len], slab[:, 0:n_len],
                                mks[:, mid * TQ:mid * TQ + n_len])
                            cur.append((t, j, slab2[:, 0:n_len],
                                        n_off, n_len))
                        else:
                            cur.append((t, j, slab[:, 0:n_len],
                                        n_off, n_len))
                    emit_avs(pending2)
                    pending2 = pending
                    pending = cur
                    yield
                emit_avs(pending2)
                emit_avs(pending)
                # normalize: yT /= denominator (row HD of y psum)
                for t in range(2):
                    den0 = work.tile([1, TQ], F32, name="den0", tag="den0",
                                     bufs=4)
                    nc.vector.tensor_copy(den0[:, :], yps[t][HD:HD + 1, :])
                    den = work.tile([1, TQ], F32, name="den", tag="den",
                                    bufs=4)
                    nc.vector.reciprocal_approx_fast(den[:, :], den0[:, :])
                    bc = work.tile([HD, TQ], F32, name="bc", tag="bc", bufs=4)
                    nc.gpsimd.partition_broadcast(bc[:, :], den[:, :])
                    nc.vector.tensor_mul(
                        yT[b][HD * t:HD * (t + 1), Q * TQ:(Q + 1) * TQ],
                        yps[t][0:HD, :], bc[:, :])

            # ---- interleaved emission -------------------------------------
            import itertools

            def chain_steps(*gens):
                for g in gens:
                    yield from g

            for _ in kqv_steps(0):
                pass
            for _ in transpose_steps(0):
                pass
            s1 = chain_steps(*[attn_steps(0, Q) for Q in range(NQ)])
            s2 = chain_steps(kqv_steps(1), transpose_steps(1))
            # round-robin: ~2 attention-b0 groups per kqv-b1 tile, so the
            # TensorE always has independent work queued behind the
            # ACT-bound attention pipeline
    
            s2_live = True
            for i in itertools.count():
                a_done = next(s1, StopIteration) is StopIteration
                if i % 2 == 1 and s2_live:
                    s2_live = next(s2, StopIteration) is not StopIteration
                if a_done:
                    break
            while s2_live:
                s2_live = next(s2, StopIteration) is not StopIteration
            for Q in range(NQ):
                for _ in attn_steps(1, Q):
                    pass

            # ---- AllToAll (head-sharded -> token-sharded) -----------------
            # Two column-half collectives: the projection on the first half
            # overlaps the second collective.
            HQ = TQ // 2
            with tc.tile_pool(name="dram", bufs=1, space="DRAM") as dram:
                a2a_in = [dram.tile([NCORES * 128, HQ], BF16,
                                    name=f"a2a_in{h}", tag=f"a2a_in{h}")
                          for h in range(2)]
                a2a_out = [dram.tile([NCORES * 128, HQ], BF16,
                                     name=f"a2a_out{h}", tag=f"a2a_out{h}")
                           for h in range(2)]
                yg = [[big.tile([128, HQ], BF16, name=f"yg{h}_{k}",
                                tag=f"yg{h}_{k}") for k in range(KC)]
                      for h in range(2)]
                for h in range(2):
                    for s in range(NCORES):
                        bb, qs = s // 4, s % 4
                        _dma(a2a_in[h][s * 128:(s + 1) * 128, :],
                             yT[bb][:, qs * TQ + h * HQ:
                                    qs * TQ + (h + 1) * HQ])
                    nc.gpsimd.collective_compute(
                        "AllToAll", mybir.AluOpType.bypass,
                        replica_groups=[list(range(NCORES))],
                        ins=[a2a_in[h].opt()], outs=[a2a_out[h].opt()])
                    for k in range(KC):
                        _dma(yg[h][k][:, :],
                             a2a_out[h][k * 128:(k + 1) * 128, :])

                    # ---- output projection for this q-half ----------------
                    for qt2 in range(2):
                        qt = 2 * h + qt2
                        for nch in range(2):
                            ps = pmm.tile([128, TQ], F32, name="proj_ps",
                                          tag="mm", bufs=2)
                            for k in range(KC):
                                nc.tensor.matmul(
                                    ps[:, :],
                                    yg[h][k][:, qt2 * 128:(qt2 + 1) * 128],
                                    wp[k][:, nch * TQ:(nch + 1) * TQ],
                                    start=(k == 0), stop=False)
                            nc.tensor.matmul(
                                ps[:, :], ones_r[0:1, 0:128],
                                bp[0:1, nch * TQ:(nch + 1) * TQ],
                                start=False, stop=True)
                            osb = work.tile([128, TQ], F32, name="osb",
                                            tag="osb", bufs=3)
                            nc.vector.tensor_copy(osb[:, :], ps[:, :])
                            _dma(out_d.ap()[qt * 128:(qt + 1) * 128,
                                            nch * TQ:(nch + 1) * TQ],
                                 osb[:, :])

    nc.compile()
    return nc


# ---------------------------------------------------------------- host glue
def _prep_in_maps(x, att_mask, w_kqv, b_kqv, w_proj, b_proj, masks):
    bf = bfloat16
    xTt = np.empty((B, NQ, KC, 128, TQ), dtype=bf)
    for b in range(B):
        xt = np.ascontiguousarray(x[b].T.astype(bf))          # [C, T]
        xTt[b] = xt.reshape(KC, 128, NQ, TQ).transpose(2, 0, 1, 3)

    wk3 = w_kqv.reshape(C, H, 3, HD)
    bk3 = b_kqv.reshape(H, 3, HD)
    in_maps = []
    for core in range(NCORES):
        hA, hB = 2 * core, 2 * core + 1
        wk_c = np.concatenate(
            [np.concatenate([wk3[:, hA, s, :], wk3[:, hB, s, :]], axis=1)
             for s in range(3)], axis=1).astype(bf)           # [1024, 384]
        bk_c = np.stack(
            [np.concatenate([bk3[hA, s], bk3[hB, s]])
             for s in range(3)], axis=1).astype(np.float32)   # [128, 3]
        im = {
            "xT0": xTt[0], "xT1": xTt[1],
            "wk": np.ascontiguousarray(wk_c),
            "bk": np.ascontiguousarray(bk_c),
            "wp": w_proj.astype(bf),
            "bp": b_proj.reshape(1, C).astype(bf),
        }
        if masks is not None:
            im["mk"] = masks.astype(bf).reshape(-1, TQ)
        in_maps.append(im)
    return in_maps


def kernel(x, att_mask, w_kqv, b_kqv, w_proj, b_proj, n_head):
    from concourse.bass_utils import run_bass_kernel_spmd

    x = np.asarray(x, dtype=np.float32)
    att_mask = np.asarray(att_mask)
    w_kqv = np.asarray(w_kqv, dtype=np.float32)
    b_kqv = np.asarray(b_kqv, dtype=np.float32)
    w_proj = np.asarray(w_proj, dtype=np.float32)
    b_proj = np.asarray(b_proj, dtype=np.float32)
    n_head = int(n_head)
    assert x.shape == (B, T, C) and n_head == H

    sched, masks = _make_schedule(att_mask)
    key = _sched_key(sched, masks)
    if key not in _BUILD_CACHE:
        _BUILD_CACHE[key] = _build(sched, masks)
    nc = _BUILD_CACHE[key]

    in_maps = _prep_in_maps(x, att_mask, w_kqv, b_kqv, w_proj, b_proj, masks)
    res = run_bass_kernel_spmd(nc, in_maps, core_ids=list(range(NCORES)))

    out = np.empty((B, T, C), dtype=np.float32)
    for core in range(NCORES):
        b, qs = core // 4, core % 4
        out[b, qs * TQ:(qs + 1) * TQ, :] = res.results[core]["out"]
    return out
